# revision 1
# baseline (speedup 1.0000x reference)
"""Trainium2 Bass kernel for nn_Attention (Bahdanau-style additive attention).

Reference computation:
    enc = encoder_outputs.transpose(1, 0, 2)            # [B, S, 2H]
    e_proj = enc @ w_e.T                                # [B, S, H]
    energy = tanh(h_proj[:, None, :] + e_proj + b)      # [B, S, H]
    att = energy @ v_w                                  # [B, S]
    out = softmax(att, axis=1)

Sharding: data-parallel over batch, 4 batch rows per core on 8 cores.
Per-core pipeline (all heavy compute in bf16 on the PE):
  - the encoder slice is DMA-transposed (xbar) from DRAM bf16 [S, 2H]
    into SBUF [128, 16, 512] tiles so the contraction dim (e) lands on
    partitions; one tile per 512 source positions
  - main matmul: psum[s-tile(128), h(512)] = sum_e enc^T chunk (the PE
    stationary, reused for both h-groups) @ w_e^T chunk; 16 e-chunks
    accumulate per bank
  - epilogue on the otherwise-idle Vector/Scalar engines:
    DVE adds the host-precomputed broadcast bias c_b = h_proj + attn_b,
    ACT applies tanh, DVE multiplies by v_w and reduces over h (free
    axis) straight into the attention logit column
  - batch row 0 ramps h-slice segments as its transposes land so the PE
    starts ~16us in; subsequent rows prefetch transposes inside the
    previous row's compute
h_proj ([32,1024] @ [1024,1024]) and the final softmax over [32, 2048]
are tiny and run on the host in fp32.
"""

import sys

try:
    import concourse.bass as bass  # noqa: F401
except ImportError:
    sys.path.insert(0, "/opt/trn_rl_repo")

import numpy as np
import ml_dtypes

import concourse.bacc as bacc
import concourse.mybir as mybir
import concourse.tile as tile
from concourse.bass_utils import run_bass_kernel_spmd

HID = 1024
BATCH = 32
SRC_LEN = 2048

N_CORES = 8
B_LOC = BATCH // N_CORES      # 4
E = 2 * HID                   # 2048
SG = 512                      # s per encoder transpose tile
N_SG = SRC_LEN // SG          # 4
N_EC = E // 128               # 16 e-chunks
N_HC = HID // 128             # 8 h-slices
N_ST = SRC_LEN // 128         # 16 s-tiles per batch row
HG = 512                      # h per psum bank
N_HG = HID // HG              # 2 h-groups

f32 = mybir.dt.float32
bf16 = mybir.dt.bfloat16

_NC_CACHE = {}


def _build():
    nc = bacc.Bacc(
        "TRN2", target_bir_lowering=False, debug=False, num_devices=N_CORES
    )
    enc = nc.declare_dram_parameter("enc", [B_LOC, SRC_LEN, E], bf16, isOutput=False)
    wT = nc.declare_dram_parameter("wT", [N_HC, 128, N_EC * 128], bf16, isOutput=False)
    cbb = nc.declare_dram_parameter("cbb", [B_LOC, 128, HID], f32, isOutput=False)
    vb = nc.declare_dram_parameter("vb", [128, HID], bf16, isOutput=False)
    # [b, p, st]: logit(b, st*128 + p)
    att = nc.declare_dram_parameter("att", [B_LOC, 128, N_ST], f32, isOutput=True)

    with tile.TileContext(nc) as tc:
        with (
            tc.tile_pool(name="const", bufs=1) as const_pool,
            tc.tile_pool(name="cbbp", bufs=2) as cbb_pool,
            tc.tile_pool(name="encT", bufs=6) as encT_pool,
            tc.tile_pool(name="tanhE", bufs=18) as te_pool,
            tc.tile_pool(name="scratch", bufs=3) as sc_pool,
            tc.tile_pool(name="attsb", bufs=1) as att_pool,
            tc.tile_pool(name="psum", bufs=5, space="PSUM") as psum_pool,
            tc.tile_pool(name="psumr", bufs=3, space="PSUM") as psumr_pool,
        ):
            w_sb = const_pool.tile([128, N_HC, N_EC, 128], bf16)
            vb_sb = const_pool.tile([128, HID], bf16)
            att_sb = att_pool.tile([128, B_LOC * N_ST], f32)

            def load_w_slice(hs):
                nc.sync.dma_start(
                    w_sb[:, hs].rearrange("p c h -> p (c h)"), wT[hs]
                )

            cbb_sbs = [None] * B_LOC

            def load_cbb(b):
                t = cbb_pool.tile([128, HID], f32, tag="cbb", name=f"cbb_{b}")
                nc.sync.dma_start(t[:], cbb[b])
                cbb_sbs[b] = t

            def transpose_group(b, sg):
                encT = encT_pool.tile(
                    [128, N_EC, SG], bf16, tag="encT", name=f"encT_{b}_{sg}"
                )
                nc.sync.dma_start(
                    out=encT[:],
                    in_=enc[b, sg * SG:(sg + 1) * SG, :],
                    transpose=True,
                )
                return encT

            # startup DMA order on the serial chain: just enough weight
            # for the first ramp segment before the first transpose
            load_w_slice(0)
            load_w_slice(1)

            # warmup tanh for the ACT LUT-table dependency
            warm = const_pool.tile([128, 1], f32)
            nc.scalar.activation(
                warm[:], w_sb[:, 0, 0, 0:1], mybir.ActivationFunctionType.Tanh
            )

            def lhs_enc(encT, st, c):
                j = st % N_SG
                return encT[:, c, j * 128:(j + 1) * 128]

            def epilogue_half(b, st, ps, hg, tanhE):
                # energy = tanh(psum + c_b), half h-group at a time
                pre = sc_pool.tile(
                    [128, HG], bf16, tag="pre", name=f"pre_{b}_{st}_{hg}"
                )
                nc.vector.tensor_add(
                    out=pre[:],
                    in0=ps[:],
                    in1=cbb_sbs[b][:, hg * HG:(hg + 1) * HG],
                )
                nc.scalar.activation(
                    tanhE[:, hg * HG:(hg + 1) * HG], pre[:],
                    mybir.ActivationFunctionType.Tanh,
                )

            def vdot(b, st, tanhE):
                # energy * v then reduce over h (free axis), both on DVE
                outj = sc_pool.tile(
                    [128, HID], bf16, tag="ttr", name=f"ttr_{b}_{st}"
                )
                nc.vector.tensor_mul(out=outj[:], in0=tanhE[:], in1=vb_sb[:])
                nc.vector.tensor_reduce(
                    att_sb[:, b * N_ST + st:b * N_ST + st + 1],
                    outj[:],
                    mybir.AxisListType.X,
                    mybir.AluOpType.add,
                )

            # ---- batch row 0: ramp as transposes land ----
            # pass 1: h-group 0 per s-tile in two h-slice-pair segments;
            # each segment is one accumulation group on the bank and is
            # drained before the next segment reopens the zero region
            encTs = []
            tanhEs = {}
            for sg in range(N_SG):
                encTs.append(transpose_group(0, sg))
                if sg == 0:
                    load_cbb(0)
                    load_w_slice(2)
                    load_w_slice(3)
                elif sg == 1:
                    for hs in range(4, 6):
                        load_w_slice(hs)
                elif sg == 2:
                    for hs in range(6, N_HC):
                        load_w_slice(hs)
            nc.sync.dma_start(vb_sb[:], vb[:])
            for sg in range(N_SG):
                for st in range(sg * N_SG, (sg + 1) * N_SG):
                    tanhEs[st] = te_pool.tile(
                        [128, HID], bf16, tag="te", name=f"te0_{st}"
                    )
                for seg in range(2):
                    for st in range(sg * N_SG, (sg + 1) * N_SG):
                        ps = psumr_pool.tile(
                            [128, 256], f32, tag="psr", name=f"psr_{st}_{seg}"
                        )
                        for c in range(N_EC):
                            for hh in range(2):
                                hs = seg * 2 + hh
                                nc.tensor.matmul(
                                    ps[:, hh * 128:(hh + 1) * 128],
                                    lhsT=lhs_enc(encTs[sg], st, c),
                                    rhs=w_sb[:, hs, c, :],
                                    start=(c == 0 and hh == 0),
                                    stop=(c == N_EC - 1 and hh == 1),
                                )
                        pre = sc_pool.tile(
                            [128, 256], bf16, tag="prer", name=f"prer_{st}_{seg}"
                        )
                        nc.vector.tensor_add(
                            out=pre[:],
                            in0=ps[:],
                            in1=cbb_sbs[0][:, seg * 256:(seg + 1) * 256],
                        )
                        nc.scalar.activation(
                            tanhEs[st][:, seg * 256:(seg + 1) * 256], pre[:],
                            mybir.ActivationFunctionType.Tanh,
                        )
            # pass 2: h-group 1 + v-dot per s-tile; prefetch b1's tiles
            encTs_next = []
            for st in range(N_ST):
                sg = st // N_SG
                ps1 = psum_pool.tile([128, HG], f32, tag="ps", name=f"ps1_{st}")
                for c in range(N_EC):
                    nc.tensor.matmul(
                        ps1[:],
                        lhsT=lhs_enc(encTs[sg], st, c),
                        rhs=w_sb[:, 4:8, c, :],
                        start=(c == 0),
                        stop=(c == N_EC - 1),
                    )
                if st == 0:
                    encTs_next.append(transpose_group(1, 0))
                    load_cbb(1)
                elif st in (2, 5, 9):
                    encTs_next.append(transpose_group(1, len(encTs_next)))
                epilogue_half(0, st, ps1, 1, tanhEs[st])
                vdot(0, st, tanhEs[st])
            nc.sync.dma_start(att[0], att_sb[:, 0:N_ST])

            # ---- batch rows 1..3: steady state ----
            for b in range(1, B_LOC):
                encTs = encTs_next
                encTs_next = []
                for st in range(N_ST):
                    sg = st // N_SG
                    if b < B_LOC - 1:
                        if st == 1:
                            encTs_next.append(transpose_group(b + 1, 0))
                            load_cbb(b + 1)
                        elif st in (3, 6, 10):
                            encTs_next.append(transpose_group(b + 1, len(encTs_next)))
                    ps = [
                        psum_pool.tile(
                            [128, HG], f32, tag="ps", name=f"ps_{b}_{st}_{g}"
                        )
                        for g in range(N_HG)
                    ]
                    for c in range(N_EC):
                        for hg in range(N_HG):
                            nc.tensor.matmul(
                                ps[hg][:],
                                lhsT=lhs_enc(encTs[sg], st, c),
                                rhs=w_sb[:, hg * 4:(hg + 1) * 4, c, :],
                                start=(c == 0),
                                stop=(c == N_EC - 1),
                            )
                    tanhE = te_pool.tile(
                        [128, HID], bf16, tag="te", name=f"te_{b}_{st}"
                    )
                    for hg in range(N_HG):
                        epilogue_half(b, st, ps[hg], hg, tanhE)
                    vdot(b, st, tanhE)
                nc.sync.dma_start(att[b], att_sb[:, b * N_ST:(b + 1) * N_ST])
    nc.compile()
    return nc


def _get_nc():
    if "nc" not in _NC_CACHE:
        _NC_CACHE["nc"] = _build()
    return _NC_CACHE["nc"]


def kernel(hidden, encoder_outputs, attn_w, attn_b, v_w, _trace=False):
    hidden = np.asarray(hidden, dtype=np.float32)
    encoder_outputs = np.asarray(encoder_outputs, dtype=np.float32)
    attn_w = np.asarray(attn_w, dtype=np.float32)
    attn_b = np.asarray(attn_b, dtype=np.float32)
    v_w = np.asarray(v_w, dtype=np.float32)

    c_b = hidden @ attn_w[:, :HID].T + attn_b          # [B, H] fp32
    w_e = attn_w[:, HID:]                              # [H, E]
    wT_bf = np.ascontiguousarray(
        w_e.reshape(N_HC, 128, N_EC, 128).transpose(0, 3, 2, 1)
        .reshape(N_HC, 128, N_EC * 128)
    ).astype(ml_dtypes.bfloat16)
    vb_dev = np.ascontiguousarray(
        np.broadcast_to(v_w[None, :], (128, HID))
    ).astype(ml_dtypes.bfloat16)

    nc = _get_nc()
    in_maps = []
    for core in range(N_CORES):
        b0 = core * B_LOC
        enc_bf = np.ascontiguousarray(
            encoder_outputs[:, b0:b0 + B_LOC, :].transpose(1, 0, 2)
        ).astype(ml_dtypes.bfloat16)
        cbb_dev = np.ascontiguousarray(
            np.broadcast_to(c_b[b0:b0 + B_LOC, None, :], (B_LOC, 128, HID))
        ).astype(np.float32)
        in_maps.append(
            {"enc": enc_bf, "wT": wT_bf, "cbb": cbb_dev, "vb": vb_dev}
        )

    res = run_bass_kernel_spmd(
        nc, in_maps, core_ids=list(range(N_CORES)), trace=_trace
    )
    if _trace:
        _NC_CACHE["last_result"] = res

    att = np.concatenate(
        [
            res.results[c]["att"].transpose(0, 2, 1).reshape(B_LOC, SRC_LEN)
            for c in range(N_CORES)
        ],
        axis=0,
    )  # [B, S] logits

    m = att.max(axis=1, keepdims=True)
    e = np.exp(att - m)
    out = e / e.sum(axis=1, keepdims=True)
    return out.astype(np.float32)



# revision 5
# speedup vs baseline: 1.6817x; 1.6817x over previous
"""Trainium2 Bass kernel for nn_Attention (Bahdanau-style additive attention).

Reference computation:
    enc = encoder_outputs.transpose(1, 0, 2)            # [B, S, 2H]
    e_proj = enc @ w_e.T                                # [B, S, H]
    energy = tanh(h_proj[:, None, :] + e_proj + b)      # [B, S, H]
    att = energy @ v_w                                  # [B, S]
    out = softmax(att, axis=1)

Sharding: data-parallel over batch, 4 batch rows per core on 8 cores.

The dominant cost is the e_proj matmul (34.4 GFLOP/core). This version
runs it in fp8 (e4m3) with DoubleRow perf mode (2 fp8 MACs per PE cell
per cycle, K=256 per accumulation chunk), ~2x the bf16 matmul rate:
  - host pre-transposes + quantizes enc to fp8 [b, ec, p, i, s] tiles
    (e = ec*256 + i*128 + p), so the kernel does plain contiguous DMAs
  - w_e is quantized to fp8 [ec, p, i, h] and kept SBUF-resident
  - per s-tile: psum[s(128), h(512)] accumulates 8 DoubleRow matmuls
    (lhsT = enc chunk [128, 2, 128] stationary, rhs = w chunk
    [128, 2, 512] moving), two h-groups = two psum banks
  - epilogue off the PE: DVE adds the (scaled) bias c_b = h_proj +
    attn_b in fp32, ACT applies tanh with a 2^-13 descale folded into
    its input scale, and a single fused DVE tensor_tensor_reduce does
    the v-weighted reduction straight into the logit column
  - fp8 values are scaled into range (enc x16, w x512, both exact
    powers of two); the product scale 2^13 is removed by the ACT scale

fp8 quantization alone would put the softmax rel-err at ~1.9e-2 --
too close to the 2e-2 gate. The host therefore subtracts the COHERENT
part of the logit error, which is exactly computable with matvecs:
  delta_att(b,s) ~= sum_h v_h * tanh'(u) * delta_u(b,s,h)
                 ~= sum_h (v_h * E[tanh'|b,h]) * delta_u
  with sum_h vt_h * delta_u = enc8_row . (w8^T vt) - enc_row . (w^T vt)
E[tanh'|b,h] is a 1D Gaussian integral (Gauss-Hermite) since
u(b,s,h) ~ N(c_b[b,h], ||w_e[h]||^2) over s. This cuts the measured
rel-err to ~5.6e-3 in simulation. h_proj and the final softmax are
tiny and run on the host in fp32.
"""

import sys

try:
    import concourse.bass as bass  # noqa: F401
except ImportError:
    sys.path.insert(0, "/opt/trn_rl_repo")

import numpy as np
import ml_dtypes

import concourse.bacc as bacc
import concourse.mybir as mybir
import concourse.tile as tile
from concourse.bass_utils import run_bass_kernel_spmd

HID = 1024
BATCH = 32
SRC_LEN = 2048

N_CORES = 8
B_LOC = BATCH // N_CORES      # 4
E = 2 * HID                   # 2048
N_EC = E // 256               # 8 e-chunks of 256 (DoubleRow K)
N_ST = SRC_LEN // 128         # 16 s-tiles per batch row
HG = 512                      # h per psum bank
N_HG = HID // HG              # 2 h-groups
N_RAMP = 4                    # s-tiles computed e-chunk-major at startup

ENC_SCALE = 16.0              # exact powers of two
W_SCALE = 512.0
INV_SC = 1.0 / (ENC_SCALE * W_SCALE)

f32 = mybir.dt.float32
bf16 = mybir.dt.bfloat16
f8 = mybir.dt.float8e4
DR = mybir.MatmulPerfMode.DoubleRow

_NC_CACHE = {}


def _build():
    nc = bacc.Bacc(
        "TRN2", target_bir_lowering=False, debug=False, num_devices=N_CORES
    )
    enc8 = nc.declare_dram_parameter(
        "enc8", [B_LOC, N_EC, 128, 2, SRC_LEN], f8, isOutput=False
    )
    w8 = nc.declare_dram_parameter("w8", [N_EC, 128, 2, HID], f8, isOutput=False)
    cbb = nc.declare_dram_parameter("cbb", [B_LOC, 128, HID], f32, isOutput=False)
    vb = nc.declare_dram_parameter("vb", [128, HID], bf16, isOutput=False)
    # [b, p, st]: logit(b, st*128 + p)
    att = nc.declare_dram_parameter("att", [B_LOC, 128, N_ST], f32, isOutput=True)

    with tile.TileContext(nc) as tc:
        with (
            tc.tile_pool(name="const", bufs=1) as const_pool,
            tc.tile_pool(name="cbbp", bufs=2) as cbb_pool,
            tc.tile_pool(name="encp", bufs=16) as enc_pool,
            tc.tile_pool(name="tanhE", bufs=4) as te_pool,
            tc.tile_pool(name="prep", bufs=4) as pre_pool,
            tc.tile_pool(name="scr", bufs=3) as sc_pool,
            tc.tile_pool(name="attsb", bufs=1) as att_pool,
            tc.tile_pool(name="psum", bufs=8, space="PSUM") as psum_pool,
        ):
            w_sb = const_pool.tile([128, N_EC, 2, HID], f8)
            vb_sb = const_pool.tile([128, HID], bf16)
            att_sb = att_pool.tile([128, B_LOC * N_ST], f32)

            enc_ts = {}
            cbb_sbs = [None] * B_LOC

            def load_w(ec):
                nc.sync.dma_start(w_sb[:, ec], w8[ec])

            def load_enc(b, ec):
                t = enc_pool.tile(
                    [128, 2, SRC_LEN], f8, tag="enc", name=f"enc_{b}_{ec}"
                )
                nc.sync.dma_start(t[:], enc8[b, ec])
                enc_ts[(b, ec)] = t

            def load_cbb(b):
                t = cbb_pool.tile([128, HID], f32, tag="cbb", name=f"cbb_{b}")
                nc.sync.dma_start(t[:], cbb[b])
                cbb_sbs[b] = t

            # startup DMAs: weight chunk then matching enc chunk so the
            # first matmuls can begin while the rest streams in
            for ec in range(N_EC):
                load_w(ec)
                load_enc(0, ec)
            load_cbb(0)
            nc.sync.dma_start(vb_sb[:], vb[:])

            # warmup tanh for the ACT LUT-table dependency
            warm = const_pool.tile([128, 1], f32)
            nc.scalar.activation(
                warm[:], vb_sb[:, 0:1], mybir.ActivationFunctionType.Tanh
            )

            def mm(ps_hg, b, st, ec):
                lhsT = enc_ts[(b, ec)][:, :, st * 128:(st + 1) * 128]
                for hg in range(N_HG):
                    nc.tensor.matmul(
                        ps_hg[hg][:],
                        lhsT=lhsT,
                        rhs=w_sb[:, ec, :, hg * HG:(hg + 1) * HG],
                        start=(ec == 0),
                        stop=(ec == N_EC - 1),
                        perf_mode=DR,
                    )

            def epilogue(b, st, ps_hg):
                te = te_pool.tile([128, HID], bf16, tag="te", name=f"te_{b}_{st}")
                for hg in range(N_HG):
                    pre = pre_pool.tile(
                        [128, HG], f32, tag="pre", name=f"pre_{b}_{st}_{hg}"
                    )
                    nc.vector.tensor_add(
                        out=pre[:],
                        in0=ps_hg[hg][:],
                        in1=cbb_sbs[b][:, hg * HG:(hg + 1) * HG],
                    )
                    nc.scalar.activation(
                        te[:, hg * HG:(hg + 1) * HG], pre[:],
                        mybir.ActivationFunctionType.Tanh,
                        scale=INV_SC,
                    )
                col = b * N_ST + st
                prod = sc_pool.tile(
                    [128, HID], bf16, tag="ttr", name=f"ttr_{b}_{st}"
                )
                nc.vector.tensor_mul(out=prod[:], in0=te[:], in1=vb_sb[:])
                nc.vector.tensor_reduce(
                    att_sb[:, col:col + 1],
                    prod[:],
                    mybir.AxisListType.X,
                    mybir.AluOpType.add,
                )

            def psum_pair(b, st):
                return [
                    psum_pool.tile([128, HG], f32, tag="ps", name=f"ps_{b}_{st}_{g}")
                    for g in range(N_HG)
                ]

            # ---- batch row 0 ramp: first N_RAMP s-tiles e-chunk-major so
            # the PE starts as soon as (w8[0], enc[0,0]) land instead of
            # waiting for the whole row
            ramp_ps = [psum_pair(0, st) for st in range(N_RAMP)]
            for ec in range(N_EC):
                for st in range(N_RAMP):
                    mm(ramp_ps[st], 0, st, ec)
            for st in range(N_RAMP):
                epilogue(0, st, ramp_ps[st])

            # ---- steady state ----
            for b in range(B_LOC):
                for st in range(0 if b else N_RAMP, N_ST):
                    # prefetch next row's tiles mid-row
                    if b < B_LOC - 1:
                        if st == 4:
                            load_cbb(b + 1)
                        if 4 <= st < 4 + N_EC:
                            load_enc(b + 1, st - 4)
                    ps_hg = psum_pair(b, st)
                    for ec in range(N_EC):
                        mm(ps_hg, b, st, ec)
                    epilogue(b, st, ps_hg)
                nc.sync.dma_start(att[b], att_sb[:, b * N_ST:(b + 1) * N_ST])
    nc.compile()
    return nc


def _get_nc():
    if "nc" not in _NC_CACHE:
        _NC_CACHE["nc"] = _build()
    return _NC_CACHE["nc"]


def kernel(hidden, encoder_outputs, attn_w, attn_b, v_w, _trace=False):
    hidden = np.asarray(hidden, dtype=np.float32)
    encoder_outputs = np.asarray(encoder_outputs, dtype=np.float32)
    attn_w = np.asarray(attn_w, dtype=np.float32)
    attn_b = np.asarray(attn_b, dtype=np.float32)
    v_w = np.asarray(v_w, dtype=np.float32)

    c_b = hidden @ attn_w[:, :HID].T + attn_b          # [B, H] fp32
    w_e = np.ascontiguousarray(attn_w[:, HID:])        # [H, E]

    # fp8 quantization (scales are exact powers of two)
    w8_q = (w_e * np.float32(W_SCALE)).astype(ml_dtypes.float8_e4m3)   # [H, E]
    e8_q = (encoder_outputs * np.float32(ENC_SCALE)).astype(
        ml_dtypes.float8_e4m3
    )                                                                   # [S, B, E]

    # device weight layout [ec, p, i, h], e = ec*256 + i*128 + p
    w8_dev = np.ascontiguousarray(
        w8_q.T.reshape(N_EC, 2, 128, HID).transpose(0, 2, 1, 3)
    )
    vb_dev = np.ascontiguousarray(
        np.broadcast_to(v_w[None, :], (128, HID))
    ).astype(ml_dtypes.bfloat16)

    nc = _get_nc()
    sc = np.float32(ENC_SCALE * W_SCALE)
    in_maps = []
    for core in range(N_CORES):
        b0 = core * B_LOC
        enc_dev = np.ascontiguousarray(
            e8_q[:, b0:b0 + B_LOC, :].transpose(1, 2, 0)
            .reshape(B_LOC, N_EC, 2, 128, SRC_LEN).transpose(0, 1, 3, 2, 4)
        )
        cbb_dev = np.ascontiguousarray(
            np.broadcast_to(
                (c_b[b0:b0 + B_LOC] * sc)[:, None, :], (B_LOC, 128, HID)
            )
        ).astype(np.float32)
        in_maps.append(
            {"enc8": enc_dev, "w8": w8_dev, "cbb": cbb_dev, "vb": vb_dev}
        )

    res = run_bass_kernel_spmd(
        nc, in_maps, core_ids=list(range(N_CORES)), trace=_trace
    )
    if _trace:
        _NC_CACHE["last_result"] = res

    att = np.concatenate(
        [
            res.results[c]["att"].transpose(0, 2, 1).reshape(B_LOC, SRC_LEN)
            for c in range(N_CORES)
        ],
        axis=0,
    ).astype(np.float32)  # [B, S] raw fp8-path logits

    # host correction: subtract the exactly-computable coherent part of
    # the fp8 quantization error, weighted by E[tanh' | b, h]
    w8_deq = w8_q.astype(np.float32) / np.float32(W_SCALE)     # [H, E]
    sig_h = np.linalg.norm(w_e, axis=1)                        # [H]
    xs, ws_gh = np.polynomial.hermite_e.hermegauss(21)
    z = sig_h[None, :, None] * xs[None, None, :] + c_b[:, :, None]
    c_bh = (np.cosh(z) ** -2 * ws_gh[None, None, :]).sum(-1) / np.sqrt(
        2 * np.pi
    )                                                          # [B, H]
    for b in range(BATCH):
        vt = (v_w * c_bh[b]).astype(np.float64)
        g8 = w8_deq.T.astype(np.float64) @ vt                  # [E]
        g0 = w_e.T.astype(np.float64) @ vt
        e8b = e8_q[:, b, :].astype(np.float64) / ENC_SCALE     # [S, E]
        encb = encoder_outputs[:, b, :].astype(np.float64)
        att[b] -= (e8b @ g8 - encb @ g0).astype(np.float32)

    m = att.max(axis=1, keepdims=True)
    e = np.exp(att - m)
    out = e / e.sum(axis=1, keepdims=True)
    return out.astype(np.float32)


# revision 8
# speedup vs baseline: 1.6914x; 1.0058x over previous
"""Trainium2 Bass kernel for nn_Attention (Bahdanau-style additive attention).

Reference computation:
    enc = encoder_outputs.transpose(1, 0, 2)            # [B, S, 2H]
    e_proj = enc @ w_e.T                                # [B, S, H]
    energy = tanh(h_proj[:, None, :] + e_proj + b)      # [B, S, H]
    att = energy @ v_w                                  # [B, S]
    out = softmax(att, axis=1)

Sharding: data-parallel over batch, 4 batch rows per core on 8 cores.

The dominant cost is the e_proj matmul (34.4 GFLOP/core). This version
runs it in fp8 (e4m3) with DoubleRow perf mode (2 fp8 MACs per PE cell
per cycle, K=256 per accumulation chunk), ~2x the bf16 matmul rate:
  - host pre-transposes + quantizes enc to fp8 [b, ec, p, i, s] tiles
    (e = ec*256 + i*128 + p), so the kernel does plain contiguous DMAs
  - w_e is quantized to fp8 [ec, p, i, h] and kept SBUF-resident
  - per s-tile: psum[s(128), h(512)] accumulates 8 DoubleRow matmuls
    (lhsT = enc chunk [128, 2, 128] stationary, rhs = w chunk
    [128, 2, 512] moving), two h-groups = two psum banks
  - epilogue off the PE: DVE adds the (scaled) bias c_b = h_proj +
    attn_b in fp32, ACT applies tanh with a 2^-13 descale folded into
    its input scale, and a single fused DVE tensor_tensor_reduce does
    the v-weighted reduction straight into the logit column
  - fp8 values are scaled into range (enc x16, w x512, both exact
    powers of two); the product scale 2^13 is removed by the ACT scale

fp8 quantization alone would put the softmax rel-err at ~1.9e-2 --
too close to the 2e-2 gate. The host therefore subtracts the COHERENT
part of the logit error, which is exactly computable with matvecs:
  delta_att(b,s) ~= sum_h v_h * tanh'(u) * delta_u(b,s,h)
                 ~= sum_h (v_h * E[tanh'|b,h]) * delta_u
  with sum_h vt_h * delta_u = enc8_row . (w8^T vt) - enc_row . (w^T vt)
E[tanh'|b,h] is a 1D Gaussian integral (Gauss-Hermite) since
u(b,s,h) ~ N(c_b[b,h], ||w_e[h]||^2) over s. This cuts the measured
rel-err to ~5.6e-3 in simulation. h_proj and the final softmax are
tiny and run on the host in fp32.
"""

import sys

try:
    import concourse.bass as bass  # noqa: F401
except ImportError:
    sys.path.insert(0, "/opt/trn_rl_repo")

import numpy as np
import ml_dtypes

import concourse.bacc as bacc
import concourse.mybir as mybir
import concourse.tile as tile
from concourse.bass_utils import run_bass_kernel_spmd

HID = 1024
BATCH = 32
SRC_LEN = 2048

N_CORES = 8
B_LOC = BATCH // N_CORES      # 4
E = 2 * HID                   # 2048
N_EC = E // 256               # 8 e-chunks of 256 (DoubleRow K)
N_ST = SRC_LEN // 128         # 16 s-tiles per batch row
HG = 512                      # h per psum bank
N_HG = HID // HG              # 2 h-groups
N_RAMP = 4                    # s-tiles computed e-chunk-major at startup
SPLIT_RAMP = True             # split row-0 enc DMAs into s-halves

ENC_SCALE = 16.0              # exact powers of two
W_SCALE = 512.0
INV_SC = 1.0 / (ENC_SCALE * W_SCALE)

f32 = mybir.dt.float32
bf16 = mybir.dt.bfloat16
f8 = mybir.dt.float8e4
DR = mybir.MatmulPerfMode.DoubleRow

_NC_CACHE = {}


def _build():
    nc = bacc.Bacc(
        "TRN2", target_bir_lowering=False, debug=False, num_devices=N_CORES
    )
    enc8 = nc.declare_dram_parameter(
        "enc8", [B_LOC, N_EC, 128, 2, SRC_LEN], f8, isOutput=False
    )
    w8 = nc.declare_dram_parameter("w8", [N_EC, 128, 2, HID], f8, isOutput=False)
    cbb = nc.declare_dram_parameter("cbb", [B_LOC, 128, HID], f32, isOutput=False)
    vb = nc.declare_dram_parameter("vb", [128, HID], bf16, isOutput=False)
    # [b, p, st]: logit(b, st*128 + p)
    att = nc.declare_dram_parameter("att", [B_LOC, 128, N_ST], f32, isOutput=True)

    with tile.TileContext(nc) as tc:
        with (
            tc.tile_pool(name="const", bufs=1) as const_pool,
            tc.tile_pool(name="cbbp", bufs=2) as cbb_pool,
            tc.tile_pool(name="encp", bufs=16) as enc_pool,
            tc.tile_pool(name="tanhE", bufs=4) as te_pool,
            tc.tile_pool(name="prep", bufs=4) as pre_pool,
            tc.tile_pool(name="scr", bufs=3) as sc_pool,
            tc.tile_pool(name="attsb", bufs=1) as att_pool,
            tc.tile_pool(name="psum", bufs=8, space="PSUM") as psum_pool,
        ):
            w_sb = const_pool.tile([128, N_EC, 2, HID], f8)
            vb_sb = const_pool.tile([128, HID], bf16)
            att_sb = att_pool.tile([128, B_LOC * N_ST], f32)

            enc_ts = {}
            cbb_sbs = [None] * B_LOC

            def load_w(ec):
                nc.sync.dma_start(w_sb[:, ec], w8[ec])

            def load_enc(b, ec, split=False):
                t = enc_pool.tile(
                    [128, 2, SRC_LEN], f8, tag="enc", name=f"enc_{b}_{ec}"
                )
                if split:
                    # first half only: covers s-tiles 0..7 for the ramp;
                    # caller issues the second half later
                    nc.sync.dma_start(
                        t[:, :, 0:SRC_LEN // 2], enc8[b, ec, :, :, 0:SRC_LEN // 2]
                    )
                else:
                    nc.sync.dma_start(t[:], enc8[b, ec])
                enc_ts[(b, ec)] = t

            def load_enc_tail(b, ec):
                t = enc_ts[(b, ec)]
                nc.sync.dma_start(
                    t[:, :, SRC_LEN // 2:], enc8[b, ec, :, :, SRC_LEN // 2:]
                )

            def load_cbb(b):
                t = cbb_pool.tile([128, HID], f32, tag="cbb", name=f"cbb_{b}")
                nc.sync.dma_start(t[:], cbb[b])
                cbb_sbs[b] = t

            # startup DMAs in e-chunk waves (w[ec] 256KB + enc-half 512KB
            # ~= the PE's per-chunk ramp consumption) so the first matmuls
            # begin after one wave and never starve through the ramp
            for ec in range(N_EC):
                load_w(ec)
                load_enc(0, ec, split=SPLIT_RAMP)
            load_cbb(0)
            nc.sync.dma_start(vb_sb[:], vb[:])
            if SPLIT_RAMP:
                for ec in range(N_EC):
                    load_enc_tail(0, ec)

            # warmup tanh for the ACT LUT-table dependency
            warm = const_pool.tile([128, 1], f32)
            nc.scalar.activation(
                warm[:], vb_sb[:, 0:1], mybir.ActivationFunctionType.Tanh
            )

            def mm(ps_hg, b, st, ec):
                lhsT = enc_ts[(b, ec)][:, :, st * 128:(st + 1) * 128]
                for hg in range(N_HG):
                    nc.tensor.matmul(
                        ps_hg[hg][:],
                        lhsT=lhsT,
                        rhs=w_sb[:, ec, :, hg * HG:(hg + 1) * HG],
                        start=(ec == 0),
                        stop=(ec == N_EC - 1),
                        perf_mode=DR,
                    )

            def epilogue(b, st, ps_hg):
                te = te_pool.tile([128, HID], bf16, tag="te", name=f"te_{b}_{st}")
                for hg in range(N_HG):
                    pre = pre_pool.tile(
                        [128, HG], f32, tag="pre", name=f"pre_{b}_{st}_{hg}"
                    )
                    nc.vector.tensor_add(
                        out=pre[:],
                        in0=ps_hg[hg][:],
                        in1=cbb_sbs[b][:, hg * HG:(hg + 1) * HG],
                    )
                    nc.scalar.activation(
                        te[:, hg * HG:(hg + 1) * HG], pre[:],
                        mybir.ActivationFunctionType.Tanh,
                        scale=INV_SC,
                    )
                col = b * N_ST + st
                prod = sc_pool.tile(
                    [128, HID], bf16, tag="ttr", name=f"ttr_{b}_{st}"
                )
                nc.vector.tensor_mul(out=prod[:], in0=te[:], in1=vb_sb[:])
                nc.vector.tensor_reduce(
                    att_sb[:, col:col + 1],
                    prod[:],
                    mybir.AxisListType.X,
                    mybir.AluOpType.add,
                )

            def psum_pair(b, st):
                return [
                    psum_pool.tile([128, HG], f32, tag="ps", name=f"ps_{b}_{st}_{g}")
                    for g in range(N_HG)
                ]

            # ---- batch row 0 ramp: first N_RAMP s-tiles e-chunk-major so
            # the PE starts as soon as (w8[0], enc[0,0]) land instead of
            # waiting for the whole row
            ramp_ps = [psum_pair(0, st) for st in range(N_RAMP)]
            for ec in range(N_EC):
                for st in range(N_RAMP):
                    mm(ramp_ps[st], 0, st, ec)
            for st in range(N_RAMP):
                epilogue(0, st, ramp_ps[st])

            # ---- steady state ----
            for b in range(B_LOC):
                for st in range(0 if b else N_RAMP, N_ST):
                    # prefetch next row's tiles mid-row
                    if b < B_LOC - 1:
                        if st == 4:
                            load_cbb(b + 1)
                        if 4 <= st < 4 + N_EC:
                            load_enc(b + 1, st - 4)
                    ps_hg = psum_pair(b, st)
                    for ec in range(N_EC):
                        mm(ps_hg, b, st, ec)
                    epilogue(b, st, ps_hg)
                nc.sync.dma_start(att[b], att_sb[:, b * N_ST:(b + 1) * N_ST])
    nc.compile()
    return nc


def _get_nc():
    if "nc" not in _NC_CACHE:
        _NC_CACHE["nc"] = _build()
    return _NC_CACHE["nc"]


def kernel(hidden, encoder_outputs, attn_w, attn_b, v_w, _trace=False):
    hidden = np.asarray(hidden, dtype=np.float32)
    encoder_outputs = np.asarray(encoder_outputs, dtype=np.float32)
    attn_w = np.asarray(attn_w, dtype=np.float32)
    attn_b = np.asarray(attn_b, dtype=np.float32)
    v_w = np.asarray(v_w, dtype=np.float32)

    c_b = hidden @ attn_w[:, :HID].T + attn_b          # [B, H] fp32
    w_e = np.ascontiguousarray(attn_w[:, HID:])        # [H, E]

    # fp8 quantization (scales are exact powers of two)
    w8_q = (w_e * np.float32(W_SCALE)).astype(ml_dtypes.float8_e4m3)   # [H, E]
    e8_q = (encoder_outputs * np.float32(ENC_SCALE)).astype(
        ml_dtypes.float8_e4m3
    )                                                                   # [S, B, E]

    # device weight layout [ec, p, i, h], e = ec*256 + i*128 + p
    w8_dev = np.ascontiguousarray(
        w8_q.T.reshape(N_EC, 2, 128, HID).transpose(0, 2, 1, 3)
    )
    vb_dev = np.ascontiguousarray(
        np.broadcast_to(v_w[None, :], (128, HID))
    ).astype(ml_dtypes.bfloat16)

    nc = _get_nc()
    sc = np.float32(ENC_SCALE * W_SCALE)
    in_maps = []
    for core in range(N_CORES):
        b0 = core * B_LOC
        enc_dev = np.ascontiguousarray(
            e8_q[:, b0:b0 + B_LOC, :].transpose(1, 2, 0)
            .reshape(B_LOC, N_EC, 2, 128, SRC_LEN).transpose(0, 1, 3, 2, 4)
        )
        cbb_dev = np.ascontiguousarray(
            np.broadcast_to(
                (c_b[b0:b0 + B_LOC] * sc)[:, None, :], (B_LOC, 128, HID)
            )
        ).astype(np.float32)
        in_maps.append(
            {"enc8": enc_dev, "w8": w8_dev, "cbb": cbb_dev, "vb": vb_dev}
        )

    res = run_bass_kernel_spmd(
        nc, in_maps, core_ids=list(range(N_CORES)), trace=_trace
    )
    if _trace:
        _NC_CACHE["last_result"] = res

    att = np.concatenate(
        [
            res.results[c]["att"].transpose(0, 2, 1).reshape(B_LOC, SRC_LEN)
            for c in range(N_CORES)
        ],
        axis=0,
    ).astype(np.float32)  # [B, S] raw fp8-path logits

    # host correction: subtract the exactly-computable coherent part of
    # the fp8 quantization error, weighted by E[tanh' | b, h]
    w8_deq = w8_q.astype(np.float32) / np.float32(W_SCALE)     # [H, E]
    sig_h = np.linalg.norm(w_e, axis=1)                        # [H]
    xs, ws_gh = np.polynomial.hermite_e.hermegauss(21)
    z = sig_h[None, :, None] * xs[None, None, :] + c_b[:, :, None]
    c_bh = (np.cosh(z) ** -2 * ws_gh[None, None, :]).sum(-1) / np.sqrt(
        2 * np.pi
    )                                                          # [B, H]
    for b in range(BATCH):
        vt = (v_w * c_bh[b]).astype(np.float64)
        g8 = w8_deq.T.astype(np.float64) @ vt                  # [E]
        g0 = w_e.T.astype(np.float64) @ vt
        e8b = e8_q[:, b, :].astype(np.float64) / ENC_SCALE     # [S, E]
        encb = encoder_outputs[:, b, :].astype(np.float64)
        att[b] -= (e8b @ g8 - encb @ g0).astype(np.float32)

    m = att.max(axis=1, keepdims=True)
    e = np.exp(att - m)
    out = e / e.sum(axis=1, keepdims=True)
    return out.astype(np.float32)


# revision 11
# speedup vs baseline: 2.0066x; 1.1864x over previous
"""Trainium2 Bass kernel for nn_Attention (Bahdanau-style additive attention).

Reference computation:
    enc = encoder_outputs.transpose(1, 0, 2)            # [B, S, 2H]
    e_proj = enc @ w_e.T                                # [B, S, H]
    energy = tanh(h_proj[:, None, :] + e_proj + b)      # [B, S, H]
    att = energy @ v_w                                  # [B, S]
    out = softmax(att, axis=1)

Sharding: data-parallel over batch, 4 batch rows per core on 8 cores.

The dominant cost is the e_proj matmul (34.4 GFLOP/core). This version
runs it in fp8 (e4m3) with DoubleRow perf mode (2 fp8 MACs per PE cell
per cycle, K=256 per accumulation chunk), ~2x the bf16 matmul rate:
  - host pre-transposes + quantizes enc to fp8 [b, ec, p, i, s] tiles
    (e = ec*256 + i*128 + p), so the kernel does plain contiguous DMAs
  - w_e is quantized to fp8 [ec, p, i, h] and kept SBUF-resident
  - per s-tile: psum[s(128), h(512)] accumulates 8 DoubleRow matmuls
    (lhsT = enc chunk [128, 2, 128] stationary, rhs = w chunk
    [128, 2, 512] moving), two h-groups = two psum banks
  - epilogue off the PE: DVE adds the (scaled) bias c_b = h_proj +
    attn_b in fp32, ACT applies tanh with a 2^-13 descale folded into
    its input scale, and a single fused DVE tensor_tensor_reduce does
    the v-weighted reduction straight into the logit column
  - fp8 values are scaled into range (enc x16, w x512, both exact
    powers of two); the product scale 2^13 is removed by the ACT scale

fp8 quantization alone would put the softmax rel-err at ~1.9e-2 --
too close to the 2e-2 gate. The host therefore subtracts the COHERENT
part of the logit error, which is exactly computable with matvecs:
  delta_att(b,s) ~= sum_h v_h * tanh'(u) * delta_u(b,s,h)
                 ~= sum_h (v_h * E[tanh'|b,h]) * delta_u
  with sum_h vt_h * delta_u = enc8_row . (w8^T vt) - enc_row . (w^T vt)
E[tanh'|b,h] is a 1D Gaussian integral (Gauss-Hermite) since
u(b,s,h) ~ N(c_b[b,h], ||w_e[h]||^2) over s. This cuts the measured
rel-err to ~5.6e-3 in simulation. h_proj and the final softmax are
tiny and run on the host in fp32.
"""

import sys

try:
    import concourse.bass as bass  # noqa: F401
except ImportError:
    sys.path.insert(0, "/opt/trn_rl_repo")

import numpy as np
import ml_dtypes

import concourse.bacc as bacc
import concourse.mybir as mybir
import concourse.tile as tile
from concourse.bass_utils import run_bass_kernel_spmd

HID = 1024
BATCH = 32
SRC_LEN = 2048

N_CORES = 8
B_LOC = BATCH // N_CORES      # 4
E = 2 * HID                   # 2048
N_EC = E // 256               # 8 e-chunks of 256 (DoubleRow K)
N_ST = SRC_LEN // 128         # 16 s-tiles per batch row
HG = 512                      # h per psum bank
N_HG = HID // HG              # 2 h-groups
N_RAMP = 4                    # s-tiles computed e-chunk-major at startup

ENC_SCALE = 16.0              # exact powers of two
W_SCALE = 512.0
INV_SC = 1.0 / (ENC_SCALE * W_SCALE)

f32 = mybir.dt.float32
bf16 = mybir.dt.bfloat16
f8 = mybir.dt.float8e4
DR = mybir.MatmulPerfMode.DoubleRow

_NC_CACHE = {}


def _build():
    nc = bacc.Bacc(
        "TRN2", target_bir_lowering=False, debug=False, num_devices=N_CORES
    )
    enc8 = nc.declare_dram_parameter(
        "enc8", [B_LOC, N_EC, 128, 2, SRC_LEN], f8, isOutput=False
    )
    w8 = nc.declare_dram_parameter("w8", [N_EC, 128, 2, HID], f8, isOutput=False)
    cbb = nc.declare_dram_parameter("cbb", [B_LOC, 128, HID], f32, isOutput=False)
    vb = nc.declare_dram_parameter("vb", [128, HID], bf16, isOutput=False)
    # [b, p, st]: logit(b, st*128 + p)
    att = nc.declare_dram_parameter("att", [B_LOC, 128, N_ST], f32, isOutput=True)

    with tile.TileContext(nc) as tc:
        with (
            tc.tile_pool(name="const", bufs=1) as const_pool,
            tc.tile_pool(name="cbbp", bufs=2) as cbb_pool,
            tc.tile_pool(name="encp", bufs=16) as enc_pool,
            tc.tile_pool(name="tanhE", bufs=4) as te_pool,
            tc.tile_pool(name="prep", bufs=4) as pre_pool,
            tc.tile_pool(name="scr", bufs=3) as sc_pool,
            tc.tile_pool(name="attsb", bufs=1) as att_pool,
            tc.tile_pool(name="psum", bufs=8, space="PSUM") as psum_pool,
        ):
            w_sb = const_pool.tile([128, N_EC, 2, HID], f8)
            vb_sb = const_pool.tile([128, HID], bf16)
            att_sb = att_pool.tile([128, B_LOC * N_ST], f32)

            enc_ts = {}
            cbb_sbs = [None] * B_LOC

            def load_w(ec, split=False):
                if split:
                    for hg in range(N_HG):
                        nc.sync.dma_start(
                            w_sb[:, ec, :, hg * HG:(hg + 1) * HG],
                            w8[ec, :, :, hg * HG:(hg + 1) * HG],
                        )
                else:
                    nc.sync.dma_start(w_sb[:, ec], w8[ec])

            def load_enc(b, ec):
                t = enc_pool.tile(
                    [128, 2, SRC_LEN], f8, tag="enc", name=f"enc_{b}_{ec}"
                )
                nc.sync.dma_start(t[:], enc8[b, ec])
                enc_ts[(b, ec)] = t

            QS = SRC_LEN // 4     # ramp DMA quarter (= s-tiles 4q..4q+3)

            def load_enc_q(b, ec, q):
                if q == 0:
                    t = enc_pool.tile(
                        [128, 2, SRC_LEN], f8, tag="enc", name=f"enc_{b}_{ec}"
                    )
                    enc_ts[(b, ec)] = t
                t = enc_ts[(b, ec)]
                nc.sync.dma_start(
                    t[:, :, q * QS:(q + 1) * QS],
                    enc8[b, ec, :, :, q * QS:(q + 1) * QS],
                )

            def load_cbb(b):
                t = cbb_pool.tile([128, HID], f32, tag="cbb", name=f"cbb_{b}")
                nc.sync.dma_start(t[:], cbb[b])
                cbb_sbs[b] = t

            # startup DMAs in small e-chunk waves (w[ec] split by h-group +
            # the first enc s-quarter, ~384KB/wave ~= the PE's per-chunk
            # ramp consumption) so the first matmul starts after one wave
            # and never starves through the ramp
            for ec in range(N_EC):
                load_w(ec, split=True)
                load_enc_q(0, ec, 0)
            load_cbb(0)
            nc.sync.dma_start(vb_sb[:], vb[:])
            for q in range(1, 4):
                for ec in range(N_EC):
                    load_enc_q(0, ec, q)

            # warmup tanh for the ACT LUT-table dependency
            warm = const_pool.tile([128, 1], f32)
            nc.scalar.activation(
                warm[:], vb_sb[:, 0:1], mybir.ActivationFunctionType.Tanh
            )

            def mm(ps_hg, b, st, ec):
                lhsT = enc_ts[(b, ec)][:, :, st * 128:(st + 1) * 128]
                for hg in range(N_HG):
                    nc.tensor.matmul(
                        ps_hg[hg][:],
                        lhsT=lhsT,
                        rhs=w_sb[:, ec, :, hg * HG:(hg + 1) * HG],
                        start=(ec == 0),
                        stop=(ec == N_EC - 1),
                        perf_mode=DR,
                    )

            def epilogue(b, st, ps_hg):
                te = te_pool.tile([128, HID], bf16, tag="te", name=f"te_{b}_{st}")
                for hg in range(N_HG):
                    pre = pre_pool.tile(
                        [128, HG], f32, tag="pre", name=f"pre_{b}_{st}_{hg}"
                    )
                    nc.vector.tensor_add(
                        out=pre[:],
                        in0=ps_hg[hg][:],
                        in1=cbb_sbs[b][:, hg * HG:(hg + 1) * HG],
                    )
                    nc.scalar.activation(
                        te[:, hg * HG:(hg + 1) * HG], pre[:],
                        mybir.ActivationFunctionType.Tanh,
                        scale=INV_SC,
                    )
                col = b * N_ST + st
                prod = sc_pool.tile(
                    [128, HID], bf16, tag="ttr", name=f"ttr_{b}_{st}"
                )
                nc.vector.tensor_mul(out=prod[:], in0=te[:], in1=vb_sb[:])
                nc.vector.tensor_reduce(
                    att_sb[:, col:col + 1],
                    prod[:],
                    mybir.AxisListType.X,
                    mybir.AluOpType.add,
                )

            def psum_pair(b, st):
                return [
                    psum_pool.tile([128, HG], f32, tag="ps", name=f"ps_{b}_{st}_{g}")
                    for g in range(N_HG)
                ]

            # ---- batch row 0 ramp: first N_RAMP s-tiles e-chunk-major so
            # the PE starts as soon as (w8[0], enc[0,0]) land instead of
            # waiting for the whole row
            ramp_ps = [psum_pair(0, st) for st in range(N_RAMP)]
            for ec in range(N_EC):
                for st in range(N_RAMP):
                    mm(ramp_ps[st], 0, st, ec)
            for st in range(N_RAMP):
                epilogue(0, st, ramp_ps[st])

            # ---- steady state ----
            for b in range(B_LOC):
                for st in range(0 if b else N_RAMP, N_ST):
                    # prefetch next row's tiles mid-row
                    if b < B_LOC - 1:
                        if st == 4:
                            load_cbb(b + 1)
                        if 4 <= st < 4 + N_EC:
                            load_enc(b + 1, st - 4)
                    ps_hg = psum_pair(b, st)
                    for ec in range(N_EC):
                        mm(ps_hg, b, st, ec)
                    epilogue(b, st, ps_hg)
                nc.sync.dma_start(att[b], att_sb[:, b * N_ST:(b + 1) * N_ST])
    nc.compile()
    return nc


def _get_nc():
    if "nc" not in _NC_CACHE:
        _NC_CACHE["nc"] = _build()
    return _NC_CACHE["nc"]


def kernel(hidden, encoder_outputs, attn_w, attn_b, v_w, _trace=False):
    hidden = np.asarray(hidden, dtype=np.float32)
    encoder_outputs = np.asarray(encoder_outputs, dtype=np.float32)
    attn_w = np.asarray(attn_w, dtype=np.float32)
    attn_b = np.asarray(attn_b, dtype=np.float32)
    v_w = np.asarray(v_w, dtype=np.float32)

    c_b = hidden @ attn_w[:, :HID].T + attn_b          # [B, H] fp32
    w_e = np.ascontiguousarray(attn_w[:, HID:])        # [H, E]

    # fp8 quantization (scales are exact powers of two)
    w8_q = (w_e * np.float32(W_SCALE)).astype(ml_dtypes.float8_e4m3)   # [H, E]
    e8_q = (encoder_outputs * np.float32(ENC_SCALE)).astype(
        ml_dtypes.float8_e4m3
    )                                                                   # [S, B, E]

    # device weight layout [ec, p, i, h], e = ec*256 + i*128 + p
    w8_dev = np.ascontiguousarray(
        w8_q.T.reshape(N_EC, 2, 128, HID).transpose(0, 2, 1, 3)
    )
    vb_dev = np.ascontiguousarray(
        np.broadcast_to(v_w[None, :], (128, HID))
    ).astype(ml_dtypes.bfloat16)

    nc = _get_nc()
    sc = np.float32(ENC_SCALE * W_SCALE)
    in_maps = []
    for core in range(N_CORES):
        b0 = core * B_LOC
        enc_dev = np.ascontiguousarray(
            e8_q[:, b0:b0 + B_LOC, :].transpose(1, 2, 0)
            .reshape(B_LOC, N_EC, 2, 128, SRC_LEN).transpose(0, 1, 3, 2, 4)
        )
        cbb_dev = np.ascontiguousarray(
            np.broadcast_to(
                (c_b[b0:b0 + B_LOC] * sc)[:, None, :], (B_LOC, 128, HID)
            )
        ).astype(np.float32)
        in_maps.append(
            {"enc8": enc_dev, "w8": w8_dev, "cbb": cbb_dev, "vb": vb_dev}
        )

    res = run_bass_kernel_spmd(
        nc, in_maps, core_ids=list(range(N_CORES)), trace=_trace
    )
    if _trace:
        _NC_CACHE["last_result"] = res

    att = np.concatenate(
        [
            res.results[c]["att"].transpose(0, 2, 1).reshape(B_LOC, SRC_LEN)
            for c in range(N_CORES)
        ],
        axis=0,
    ).astype(np.float32)  # [B, S] raw fp8-path logits

    # host correction: subtract the exactly-computable coherent part of
    # the fp8 quantization error, weighted by E[tanh' | b, h]
    w8_deq = w8_q.astype(np.float32) / np.float32(W_SCALE)     # [H, E]
    sig_h = np.linalg.norm(w_e, axis=1)                        # [H]
    xs, ws_gh = np.polynomial.hermite_e.hermegauss(21)
    z = sig_h[None, :, None] * xs[None, None, :] + c_b[:, :, None]
    c_bh = (np.cosh(z) ** -2 * ws_gh[None, None, :]).sum(-1) / np.sqrt(
        2 * np.pi
    )                                                          # [B, H]
    for b in range(BATCH):
        vt = (v_w * c_bh[b]).astype(np.float64)
        g8 = w8_deq.T.astype(np.float64) @ vt                  # [E]
        g0 = w_e.T.astype(np.float64) @ vt
        e8b = e8_q[:, b, :].astype(np.float64) / ENC_SCALE     # [S, E]
        encb = encoder_outputs[:, b, :].astype(np.float64)
        att[b] -= (e8b @ g8 - encb @ g0).astype(np.float32)

    m = att.max(axis=1, keepdims=True)
    e = np.exp(att - m)
    out = e / e.sum(axis=1, keepdims=True)
    return out.astype(np.float32)
